# revision 41
# baseline (speedup 1.0000x reference)
"""Bass/Trainium2 kernel for nn_DotProductAttention_47528108097846.

reference:
    scores = einsum('bhqd,bhkd->bhqk', Q, K) / 16
    attn = softmax(scores, axis=-1)
    h = einsum('bhqk,bhkd->bhqd', attn, V)
    return reshape(h, (S, B, H, D))

B=2, H=8, S=4096, D=64. 16 (b,h) pairs sharded as 2 per NeuronCore across 8
cores (batch+head parallel, no cross-core comms).

Measured on trn2: 304.6 us (NTFF), l2 rel err 1.12e-2 (gate 2e-2);
baseline (all-ScalarE exp, padded-128 QK) was 335.8 us.

Design (vs the baseline):
  - QK 2x2 tiling: contraction is only D=64, so each kb-pair runs as four
    concurrent 64x64 PE tiles (kb parity -> array row halves via the packed
    kt layout, k-position halves -> column groups; Q^T duplicated into
    partitions 64:127). ~1.9x QK throughput.
  - The exp bottleneck (ScalarE ACTIVATE, 284us busy in the baseline) is
    split: 6/16 of score tiles are exponentiated on the otherwise-idle
    VectorE via a one-op Schraudolph bit-trick: tensor_scalar computes
    (A*s + B)/2^16 with int16 output, whose bits ARE the bf16 pattern of
    exp(s/16); the AV matmul reads it via a zero-cost bitcast. HW-verified
    (rms 1.8%/elt); end-to-end l2 err 1.12e-2.
  - Scheduling: the PE queue is strict FIFO, so AV matmuls (which wait on
    their tile's exp) are emitted SIX pairs behind their QKs, and the
    q-group epilogue is deferred into the next q-group's pair loop --
    otherwise every stalled instruction serializes the two exp engines
    through the PE. DVE-exp tiles are Bresenham-interleaved with ScalarE
    tiles to keep PE duty high (HAM stays at 2.4 GHz; clustered tiles
    caused 32 re-throttle events).
  - Epilogue: batched transposes into one [128,4,65] psum tile, one
    strided reciprocal, broadcast tensor_tensor multiply.
"""
import numpy as np

import concourse.bass as bass
import concourse.bacc as bacc
import concourse.tile as tile
from concourse import mybir
from concourse.masks import make_identity
from concourse.bass_utils import run_bass_kernel_spmd

B, H, S, D = 2, 8, 4096, 64
N_CORES = 8
PAIRS_PER_CORE = (B * H) // N_CORES  # 2 heads per core

f32 = mybir.dt.float32
i16 = mybir.dt.int16
i32 = mybir.dt.int32
bf16 = mybir.dt.bfloat16

QG = 512             # q-group width
NQG = S // QG        # 8 q-groups per head
NKB = S // 128       # 32 k-blocks per head
NPAIR = NKB // 2     # 16 row-tiled kb pairs

# Schraudolph exp constants (scale 1/16 folded in), pre-divided by 2^16 so
# the int16-converted result IS the bf16 bit pattern of exp(s/16) (int16
# rounding of (A*s+B)/2^16 == bf16 RN rounding of the schraudolph fp32).
A_EXP = float(2.0**23 / (np.log(2.0) * 16.0) / 65536.0)
B_EXP = float((127.0 * 2**23 - 486411.0) / 65536.0)
# pairs (of 16 per q-group) whose exp runs on VectorE instead of ScalarE
DVE_M = 6


def build_attention(nc, tc, q, k, v, o, dve_m=DVE_M, repeat_loop=None):
    """Emit attention for PAIRS_PER_CORE heads.

    q/k/v/o: DRAM APs of shape [PAIRS_PER_CORE, S, D] (fp32).
    """
    import contextlib
    ctx = contextlib.ExitStack()
    consts = ctx.enter_context(tc.tile_pool(name="consts", bufs=1))
    nat = ctx.enter_context(tc.tile_pool(name="nat", bufs=2))
    persist = ctx.enter_context(tc.tile_pool(name="persist", bufs=1))
    sb = ctx.enter_context(tc.tile_pool(name="sb", bufs=3))
    pool_e = ctx.enter_context(tc.tile_pool(name="sb_e", bufs=8))
    pool_i = ctx.enter_context(tc.tile_pool(name="sb_i", bufs=6))
    pool_s = ctx.enter_context(tc.tile_pool(name="ps_s", bufs=3, space="PSUM"))
    pool_o = ctx.enter_context(tc.tile_pool(name="ps_o", bufs=1, space="PSUM"))
    pool_t = ctx.enter_context(tc.tile_pool(name="ps_t", bufs=1, space="PSUM"))

    ident = consts.tile([128, 128], f32)
    make_identity(nc, ident)
    identb = consts.tile([128, 128], bf16)
    nc.vector.tensor_copy(out=identb, in_=ident)

    # ---------------- prologue: load + transpose Q,K; build V' ----------
    # kt packed for row tiling: [0:64, i, :] = K^T block 2i,
    #                           [64:128, i, :] = K^T block 2i+1.
    # qt duplicated: [0:64] = Q^T, [64:128] = copy (both row-halves of the
    # PE array stream the same q data).
    qts, kts, v1s = [], [], []

    def emit_prologue(h):
        qt = persist.tile([128, NKB, 128], bf16, tag=f"qt{h}")
        kt = persist.tile([128, NPAIR, 128], bf16, tag=f"kt{h}")
        v1 = persist.tile([128, NKB, 128], bf16, tag=f"v1{h}")
        qts.append(qt)
        kts.append(kt)
        v1s.append(v1)

        CH = 8
        for g in range(NKB // CH):
            # K chunk: 8 s-blocks -> transposes packed into partition halves
            natc = nat.tile([128, CH, 64], f32, tag="nat")
            nc.sync.dma_start(
                out=natc,
                in_=k[h].rearrange("(n p) d -> p n d", p=128)[
                    :, g * CH:(g + 1) * CH, :])
            natbc = nat.tile([128, CH, 64], bf16, tag="natb")
            nc.vector.tensor_copy(out=natbc, in_=natc)
            ps_tr = pool_t.tile([128, CH // 2, 128], bf16, tag="t")
            for j in range(CH):
                half = j % 2
                dst = ps_tr[64 * half:64 * half + 64, j // 2, :]
                nc.tensor.transpose(dst, natbc[:, j, :], identb)
            nc.vector.tensor_copy(
                out=kt[:, g * (CH // 2):(g + 1) * (CH // 2), :], in_=ps_tr)

            # Q chunk
            natq = nat.tile([128, CH, 64], f32, tag="nat")
            nc.sync.dma_start(
                out=natq,
                in_=q[h].rearrange("(n p) d -> p n d", p=128)[
                    :, g * CH:(g + 1) * CH, :])
            natbq = nat.tile([128, CH, 64], bf16, tag="natb")
            nc.vector.tensor_copy(out=natbq, in_=natq)
            ps_tq = pool_t.tile([64, CH, 128], bf16, tag="t")
            for j in range(CH):
                nc.tensor.transpose(ps_tq[:, j, :], natbq[:, j, :], identb)
            nc.vector.tensor_copy(
                out=qt[0:64, g * CH:(g + 1) * CH, :], in_=ps_tq)

            if g == 2:
                # V' build deferred so its DVE copies don't delay startup
                nc.gpsimd.memset(v1[:, :, 64:65], 1.0)
                nc.gpsimd.memset(v1[:, :, 65:128], 0.0)
                vnat = nat.tile([128, NKB, 64], f32, tag="vnat")
                nc.sync.dma_start(
                    out=vnat, in_=v[h].rearrange("(n p) d -> p n d", p=128))
                nc.vector.tensor_copy(out=v1[:, :, 0:64], in_=vnat)

        # duplicate Q^T into the upper partition half (SBUF->SBUF DMA)
        nc.sync.dma_start(out=qt[64:128, :, :], in_=qt[0:64, :, :])

    emit_prologue(0)
    defer_prologues = repeat_loop is None
    if not defer_prologues:
        for h in range(1, PAIRS_PER_CORE):
            emit_prologue(h)

    # ---------------- main loops --------------------------------------
    def main_compute():
        # q-group epilogue, deferred into the NEXT q-group's pair loop so its
        # oT-copy-gated PE transposes don't stall the FIFO at the boundary
        # (measured ~1.6us exp-stream gap at every 16th tile without this)
        def emit_epilogue(ps_o_, out_r_, qg_):
            oT = sb.tile([65, QG], f32, tag="oT")
            nc.vector.tensor_copy(out=oT, in_=ps_o_[0:65, :])
            ps_t = pool_t.tile([128, QG // 128, 65], f32, tag="t")
            for c in range(QG // 128):
                nc.tensor.transpose(
                    ps_t[:, c, :], oT[:, c * 128:(c + 1) * 128],
                    ident[0:65, 0:65])
            rcp = sb.tile([128, QG // 128], f32, tag="rcp")
            nc.vector.reciprocal(out=rcp, in_=ps_t[:, :, 64])
            out_sb = sb.tile([128, QG // 128, 64], f32, tag="out")
            nc.vector.tensor_tensor(
                out=out_sb, in0=ps_t[:, :, 0:64],
                in1=rcp[:, :, None].to_broadcast([128, QG // 128, 64]),
                op=mybir.AluOpType.mult)
            nc.sync.dma_start(
                out=out_r_[:, qg_ * (QG // 128):(qg_ + 1) * (QG // 128), :],
                in_=out_sb)

        pending_epi = [None]
        for h in range(PAIRS_PER_CORE):
            qt, kt, v1 = qts[h], kts[h], v1s[h]
            qtf_lo = qt.rearrange("p n d -> p (n d)")[0:64, :]
            qtf_hi = qt.rearrange("p n d -> p (n d)")[64:128, :]
            out_r = o[h].rearrange("(n p) d -> p n d", p=128)
            for qg in range(NQG):
                ps_o = pool_o.tile([128, QG], f32, tag="o")
                cols = slice(qg * QG, (qg + 1) * QG)

                def av(prev_eT, prev_pair):
                    for half in range(2):
                        kb = 2 * prev_pair + half
                        nc.tensor.matmul(
                            out=ps_o,
                            lhsT=v1[:, kb, :],
                            rhs=prev_eT[:, half * QG:(half + 1) * QG],
                            start=(kb == 0), stop=(kb == NKB - 1))

                # AV is deferred SIX pairs behind QK: the PE engine queue is
                # strict FIFO, so an AV whose exp isn't ready yet blocks the
                # next pairs' (ready) QKs behind it and serializes the two
                # exp engines against each other through the PE. Depth 4 makes
                # the engines' own throughput the binding constraint.
                pend = []
                for i in range(NPAIR):
                    ps_s = pool_s.tile([128, 2 * QG], f32, tag="s")
                    # 2x2 tiled pair: kb=2i from array rows 0:64 (cols 0:QG),
                    # kb=2i+1 from rows 64:128 (cols QG:2QG); each kb's
                    # 128 k-positions split across 2 col-tiles, all four
                    # 64x64 tiles run concurrently on separate XBUSes
                    for rh, (ktr, qtr, csl) in enumerate(
                            [(kt[0:64, i, :], qtf_lo, slice(0, QG)),
                             (kt[64:128, i, :], qtf_hi, slice(QG, 2 * QG))]):
                        for ch in range(2):
                            nc.tensor.matmul(
                                out=ps_s[64 * ch:64 * ch + 64, csl],
                                lhsT=ktr[:, 64 * ch:64 * ch + 64],
                                rhs=qtr[:, cols], start=True, stop=True)
                    if i == 2 and pending_epi[0] is not None:
                        emit_epilogue(*pending_epi[0])
                        pending_epi[0] = None
                    if len(pend) >= 6:
                        av(*pend.pop(0))
                    # Bresenham spread so DVE-exp tiles interleave with
                    # ScalarE tiles (clustering starves the PE and HAM
                    # re-throttles it to 1.2 GHz)
                    if (i * dve_m) % NPAIR < dve_m:
                        # Schraudolph exp on VectorE: single op, int16 out
                        # bitcast to bf16 (verified on HW in micro_test3)
                        it = pool_i.tile([128, 2 * QG], i16, tag="i")
                        nc.vector.tensor_scalar(
                            out=it, in0=ps_s, scalar1=A_EXP, scalar2=B_EXP,
                            op0=mybir.AluOpType.mult, op1=mybir.AluOpType.add)
                        eT = it.bitcast(bf16)
                    else:
                        eT = pool_e.tile([128, 2 * QG], bf16, tag="exp")
                        nc.scalar.activation(
                            out=eT, in_=ps_s,
                            func=mybir.ActivationFunctionType.Exp,
                            scale=1.0 / 16.0)
                    pend.append((eT, i))
                while pend:
                    av(*pend.pop(0))
                pending_epi[0] = (ps_o, out_r, qg)
                if defer_prologues and h == 0 and qg == 0:
                    for h2 in range(1, PAIRS_PER_CORE):
                        emit_prologue(h2)
        emit_epilogue(*pending_epi[0])

    if repeat_loop is None:
        main_compute()
    else:
        with tc.For_i(0, repeat_loop, 1) as _:
            main_compute()
    ctx.close()


_CACHED = {}


def build_program(dve_m=DVE_M, repeat_loop=None):
    key = (dve_m, repeat_loop)
    if key in _CACHED:
        return _CACHED[key]
    nc = bacc.Bacc("TRN2", target_bir_lowering=False, debug=False,
                   num_devices=N_CORES)
    q = nc.dram_tensor("q", [PAIRS_PER_CORE, S, D], f32,
                       kind="ExternalInput").ap()
    k = nc.dram_tensor("k", [PAIRS_PER_CORE, S, D], f32,
                       kind="ExternalInput").ap()
    v = nc.dram_tensor("v", [PAIRS_PER_CORE, S, D], f32,
                       kind="ExternalInput").ap()
    o = nc.dram_tensor("o", [PAIRS_PER_CORE, S, D], f32,
                       kind="ExternalOutput").ap()
    with tile.TileContext(nc) as tc:
        build_attention(nc, tc, q, k, v, o, dve_m=dve_m,
                        repeat_loop=repeat_loop)
    nc.compile()
    _CACHED[key] = nc
    return nc


def kernel(queries, keys, values, adj=None, **_unused):
    """Full-input attention on 8 NeuronCores. Returns [S, B, H, D] fp32."""
    queries = np.ascontiguousarray(queries, dtype=np.float32)
    keys = np.ascontiguousarray(keys, dtype=np.float32)
    values = np.ascontiguousarray(values, dtype=np.float32)

    nc = build_program()
    qf = queries.reshape(B * H, S, D)
    kf = keys.reshape(B * H, S, D)
    vf = values.reshape(B * H, S, D)
    in_maps = []
    for c in range(N_CORES):
        sl = slice(c * PAIRS_PER_CORE, (c + 1) * PAIRS_PER_CORE)
        in_maps.append({"q": qf[sl], "k": kf[sl], "v": vf[sl]})
    res = run_bass_kernel_spmd(nc, in_maps, list(range(N_CORES)))
    hout = np.empty((B * H, S, D), dtype=np.float32)
    for c in range(N_CORES):
        hout[c * PAIRS_PER_CORE:(c + 1) * PAIRS_PER_CORE] = res.results[c]["o"]
    return hout.reshape(B, H, S, D).reshape(S, B, H, D)
